# revision 36
# baseline (speedup 1.0000x reference)
"""Trainium2 Bass kernel for nn_Attention_46454366273781 (sparse_attention).

Reference computation (T=2048, B=32, N=1024, H=8, K=128, K2=16):
    X = einsum('tbn,hkn->bthk', hyp, Wmh) + bmh          # per-head projections
    m = X.mean(axis=1)                                   # mean over time
    g = tanh(X @ W.T + bW) * tanh(m @ Wm.T + bWm)[:,None]
    s = g @ Wh + bWh ; a = softmax(s, axis=time)
    c = einsum('bth,bthk->bhk', a, X) ; out = c.reshape(B, H*K)

Key algebra: X itself is never needed on device.
  * scoring:  X @ W.T + bW  =  hyp @ WS.T + sbias  with WS = W @ Wmh (per head)
    and sbias = bSp + WS @ mean_t(hyp)  (scoring split around the time-mean)
  * gate:     tanh(m @ Wm.T + bWm) = tanh(WSm @ mean_t(hyp) + bSm) -- depends
    only on the time-mean, so it is a tiny per-batch vector.
  * output:   softmax weights sum to 1, so with x_bar = mean_t(hyp):
        c = [x_bar + sum_t (a_t - 1/T) hyp_t] @ Wmh^T + bmh
    The x_bar part (plus bias) is exact; the device only computes the small
    deviation term with zero-sum weights w = S*(p/Z - 1/T), which kills the
    systematic component of fp8 rounding noise.

hyp is streamed in fp8e4 in BOTH layouts (N-major for the rank-128 scoring
matmul, T-major for the deviation-weighted time reduction), with DoubleRow
fp8 matmuls (2 contraction rows/partition).  Per-core HBM traffic ~17MB,
making the kernel DMA-bound at ~360 GB/s.  Sharding: data-parallel over
batch B across 8 cores (4 batches/core).  bWh cancels inside the softmax.
"""

import numpy as np
import ml_dtypes

T, B, N, H = 2048, 32, 1024, 8
K, K2 = 128, 16          # per-head dim, attention hidden per head
NCORES = 8
BL = B // NCORES         # batches per core
NCH = N // 128           # contraction chunks over N
T128 = T // 128          # 128-sized time chunks
TC = 512                 # time chunk for scoring matmul free dim
TCH = T // TC            # time chunks (scoring)
S = 65536.0              # deviation-weight scale (keeps w in fp8 normal range)
HNU = 6                  # hN u-chunks loaded by DMA; the rest are produced
                         # on-chip by PE-transposing the resident hT tiles
WS_SCALE = 32.0          # WST prescale: WS values (std ~0.01) sit below fp8's
WMH_SCALE = 64.0         # min normal 2^-6; HW flushes fp8 subnormals to zero,
V_SCALE = 1.0 / 16.0     # so every fp8 tensor is kept in the normal range via
                         # power-of-2 prescales that are divided back out.

_cache = {}


def _build_nc():
    import concourse.mybir as mybir
    import concourse.tile as tile
    from concourse import bacc
    from concourse.masks import make_identity

    f8 = mybir.dt.float8e4
    bf16 = mybir.dt.bfloat16
    f32 = mybir.dt.float32
    AF = mybir.ActivationFunctionType
    AX = mybir.AxisListType
    OP = mybir.AluOpType
    DR = mybir.MatmulPerfMode.DoubleRow

    nc = bacc.Bacc("TRN2")
    hypT_d = nc.dram_tensor("hypT", (BL, N, T), f8, kind="ExternalInput")
    hypN_d = nc.dram_tensor("hypN", (BL, T, N), f8, kind="ExternalInput")
    WST_d = nc.dram_tensor("WST", (128, NCH, 128), f8, kind="ExternalInput")
    whD_d = nc.dram_tensor("whD", (K, H), bf16, kind="ExternalInput")
    wmhT_d = nc.dram_tensor("wmhT", (128, H, NCH, K), f8, kind="ExternalInput")
    # aux packs sbias [0:BL], mw [BL:2BL], mbarT [2BL + bl*H + h] per column
    aux_d = nc.dram_tensor("aux", (128, 2 * BL + BL * H), f32,
                           kind="ExternalInput")
    out_d = nc.dram_tensor("out", (K, BL, H), f32, kind="ExternalOutput")

    with tile.TileContext(nc) as tc, \
         tc.tile_pool(name="wpool", bufs=1) as wpool, \
         tc.tile_pool(name="hTp", bufs=3) as hTp, \
         tc.tile_pool(name="hNp", bufs=4) as hNp, \
         tc.tile_pool(name="g1p", bufs=3) as g1p, \
         tc.tile_pool(name="g2p", bufs=3) as g2p, \
         tc.tile_pool(name="seqp", bufs=2) as seqp, \
         tc.tile_pool(name="smallp", bufs=2) as smallp, \
         tc.tile_pool(name="psA", bufs=2, space="PSUM") as psA, \
         tc.tile_pool(name="psS", bufs=1, space="PSUM") as psS, \
         tc.tile_pool(name="psTp", bufs=2, space="PSUM") as psTp, \
         tc.tile_pool(name="psWTp", bufs=1, space="PSUM") as psWTp, \
         tc.tile_pool(name="psVp", bufs=1, space="PSUM") as psVp, \
         tc.tile_pool(name="psCp", bufs=1, space="PSUM") as psCp:

        # All loads go on the sync/HWDGE queue in explicit program order.
        # The scoring inputs (hT) are front-loaded so the last batch's long
        # scoring->softmax chain overlaps earlier DMA; only its hN (consumed
        # by the short v-pass) arrives last.  Stream order:
        #   hT0, aux, whD, WST, hT1, hN0, hT2, wmhT, hT3, hN1, hN2, hN3
        WST = wpool.tile([128, NCH, 128], f8)
        whD = wpool.tile([K, H], bf16)
        aux_sb = wpool.tile([128, 2 * BL + BL * H], f32)
        wmhT = wpool.tile([128, H, NCH, K], f8)
        ident = wpool.tile([128, 128], bf16)
        make_identity(nc, ident)
        identf8 = wpool.tile([128, 128], f8)
        make_identity(nc, identf8)
        c2all = wpool.tile([K, BL, H], f32)

        hTs = [hTp.tile([128, NCH, T], f8, tag="hT", name=f"hT_{i}")
               for i in range(BL)]
        hNs = [hNp.tile([128, T128, N], f8, tag="hN", name=f"hN_{i}")
               for i in range(BL)]
        TH = T // 2

        def load_hT(i):
            nc.sync.dma_start(
                out=hTs[i], in_=hypT_d[i].rearrange("(c p) t -> p c t", p=128))

        def load_hN(i):
            # only the first HNU time-chunks come from HBM; u >= HNU are
            # transposed on-chip from the hT tiles
            nc.sync.dma_start(
                out=hNs[i][:, :HNU, :],
                in_=hypN_d[i, :HNU * 128, :].rearrange("(u p) n -> p u n",
                                                       p=128))

        load_hT(0)
        nc.sync.dma_start(out=aux_sb, in_=aux_d[:])
        nc.sync.dma_start(out=whD, in_=whD_d[:])
        nc.sync.dma_start(out=WST, in_=WST_d[:])
        load_hT(1)
        load_hN(0)
        load_hT(2)
        load_hN(1)
        nc.sync.dma_start(out=wmhT, in_=wmhT_d[:])
        load_hT(3)
        load_hN(2)
        load_hN(3)

        for bl in range(BL):
            hT = hTs[bl]
            hN = hNs[bl]

            # ---- scoring: s = whD^T (tanh(WS hyp^T + sbias) * mw) ----
            s_exp = seqp.tile([8, T], f32, tag="s_exp", name=f"s_exp_{bl}")
            ssum_parts = smallp.tile([8, TCH], f32, tag="ssp", name=f"ssp_{bl}")
            for tci in range(TCH):
                tsl = slice(tci * TC, (tci + 1) * TC)
                ps = psA.tile([128, TC], f32, tag="psA", name=f"psA_{bl}_{tci}")
                for c in range(NCH // 2):
                    nc.tensor.matmul(ps, lhsT=WST[:, 2 * c:2 * c + 2, :],
                                     rhs=hT[:, 2 * c:2 * c + 2, tsl],
                                     start=(c == 0), stop=(c == NCH // 2 - 1),
                                     perf_mode=DR)
                g1 = g1p.tile([128, TC], bf16, tag="g1", name=f"g1_{bl}_{tci}")
                nc.scalar.activation(out=g1, in_=ps, func=AF.Tanh,
                                     scale=1.0 / WS_SCALE,
                                     bias=aux_sb[:, bl:bl + 1])
                g2 = g2p.tile([128, TC], bf16, tag="g2", name=f"g2_{bl}_{tci}")
                nc.vector.tensor_scalar_mul(g2, g1, aux_sb[:, BL + bl:BL + bl + 1])
                ps_s = psS.tile([8, TC], f32, tag="psS", name=f"psS_{bl}_{tci}")
                nc.tensor.matmul(ps_s, lhsT=whD, rhs=g2, start=True, stop=True)
                nc.scalar.activation(out=s_exp[:, tsl], in_=ps_s, func=AF.Exp,
                                     accum_out=ssum_parts[:, tci:tci + 1])

            # ---- zero-sum deviation weights w = S*(p/Z - 1/T) ----
            # ---- build hN u-chunks >= HNU by transposing hT on the PE;
            # ---- fills the PE while the softmax chain drains ----
            for ub in range(HNU // 2, T128 // 2):
                psT = psTp.tile([128, 2, N], f8, tag="psT",
                                name=f"psT_{bl}_{ub}")
                for j in range(2):
                    u = 2 * ub + j
                    for c in range(NCH):
                        nc.tensor.matmul(
                            psT[:, j, c * 128:(c + 1) * 128],
                            lhsT=hT[:, c, u * 128:(u + 1) * 128],
                            rhs=identf8, is_transpose=True,
                            skip_group_check=True)
                eng = (nc.vector, nc.gpsimd, nc.vector, nc.gpsimd,
                       nc.gpsimd)[ub - HNU // 2]
                eng.tensor_copy(out=hN[:, 2 * ub:2 * ub + 2, :], in_=psT)

            # wt = (p - Z/T) * (S/Z): the subtraction uses Z exactly so the
            # weights stay zero-sum even though the HW reciprocal is
            # approximate -- a reciprocal error then only scales the small
            # deviation term instead of leaking the full mean into c.
            ssum = smallp.tile([8, 1], f32, tag="ssum", name=f"ssum_{bl}")
            nc.vector.reduce_sum(out=ssum, in_=ssum_parts, axis=AX.X)
            zs = smallp.tile([8, 1], f32, tag="zs", name=f"zs_{bl}")
            nc.vector.tensor_scalar_mul(zs, ssum, 1.0 / S)
            sinvS = smallp.tile([8, 1], f32, tag="sinvS", name=f"sinvS_{bl}")
            nc.vector.reciprocal(sinvS, zs)
            zT = smallp.tile([8, 1], f32, tag="zT", name=f"zT_{bl}")
            nc.vector.tensor_scalar_mul(zT, ssum, 1.0 / T)
            negb = smallp.tile([8, 1], f32, tag="negb", name=f"negb_{bl}")
            nc.vector.tensor_scalar(out=negb, in0=zT, scalar1=sinvS,
                                    scalar2=-1.0, op0=OP.mult, op1=OP.mult)
            wt = seqp.tile([8, T], bf16, tag="wt", name=f"wt_{bl}")
            # split across ACT and DVE: this sits on the tail critical path
            nc.scalar.activation(out=wt[:, :TH], in_=s_exp[:, :TH],
                                 func=AF.Identity, scale=sinvS, bias=negb)
            nc.vector.tensor_scalar(out=wt[:, TH:], in0=s_exp[:, TH:],
                                    scalar1=zT, scalar2=sinvS, op0=OP.subtract,
                                    op1=OP.mult)

            # ---- transpose w to [t, h] and cast fp8 ----
            psWT = psWTp.tile([128, T128, 8], bf16, tag="psWT", name=f"psWT_{bl}")
            for u in range(T128):
                nc.tensor.matmul(psWT[:, u, :],
                                 lhsT=wt[:, u * 128:(u + 1) * 128],
                                 rhs=ident[:8, :8], is_transpose=True,
                                 skip_group_check=True)
            wt8T = smallp.tile([128, T128, 8], f8, tag="wt8T", name=f"wt8T_{bl}")
            nc.vector.tensor_copy(out=wt8T, in_=psWT)

            # ---- deviation v^T[n, h] = sum_t hyp[t, n] w[t, h] ----
            # n outer: each psV accumulation group must fully complete before
            # the next group's start=True, which pending-zeroes the whole
            # 2KB PSUM region and would wipe other groups' partial sums.
            psV = psVp.tile([128, NCH, 8], f32, tag="psV", name=f"psV_{bl}")
            for n in range(NCH):
                nsl = slice(n * 128, (n + 1) * 128)
                for u in range(T128 // 2):
                    nc.tensor.matmul(psV[:, n, :],
                                     lhsT=hN[:, 2 * u:2 * u + 2, nsl],
                                     rhs=wt8T[:, 2 * u:2 * u + 2, :],
                                     start=(u == 0), stop=(u == T128 // 2 - 1),
                                     perf_mode=DR, skip_group_check=True)
            vT8 = smallp.tile([128, NCH, 8], f8, tag="vT8", name=f"vT8_{bl}")
            nc.scalar.activation(out=vT8[:, :NCH // 2, :],
                                 in_=psV[:, :NCH // 2, :], func=AF.Copy,
                                 scale=V_SCALE)
            nc.scalar.activation(out=vT8[:, NCH // 2:, :],
                                 in_=psV[:, NCH // 2:, :], func=AF.Copy,
                                 scale=V_SCALE)

            # ---- c_dev^T = Wmh v_dev ; c = c_dev/S + mbar ----
            # h outer for the same reason: complete each column's group first
            ps_c = psCp.tile([128, H], f32, tag="ps_c", name=f"ps_c_{bl}")
            for h in range(H):
                for n in range(NCH // 2):
                    nc.tensor.matmul(ps_c[:, h:h + 1],
                                     lhsT=wmhT[:, h, 2 * n:2 * n + 2, :],
                                     rhs=vT8[:, 2 * n:2 * n + 2, h:h + 1],
                                     start=(n == 0), stop=(n == NCH // 2 - 1),
                                     perf_mode=DR, skip_group_check=True)
            nc.vector.scalar_tensor_tensor(
                out=c2all[:, bl, :], in0=ps_c,
                scalar=1.0 / (S * WMH_SCALE * V_SCALE),
                in1=aux_sb[:, 2 * BL + bl * H:2 * BL + (bl + 1) * H],
                op0=OP.mult, op1=OP.add)

        nc.sync.dma_start(out=out_d[:], in_=c2all)

    nc.compile()
    return nc


def _prep_inputs(hyp, Wmh, bmh, W, bW, Wm, bWm, Wh, bWh):
    """Host-side sharding + layout prep (numpy only)."""
    f8 = ml_dtypes.float8_e4m3
    bf = ml_dtypes.bfloat16
    hyp = np.asarray(hyp, np.float32)
    Wmh = np.asarray(Wmh, np.float32)
    bmh = np.asarray(bmh, np.float32)
    W = np.asarray(W, np.float32)
    bW = np.asarray(bW, np.float32)
    Wm = np.asarray(Wm, np.float32)
    bWm = np.asarray(bWm, np.float32)
    Wh = np.asarray(Wh, np.float32)

    hyp_b = np.ascontiguousarray(hyp.transpose(1, 0, 2))          # (B, T, N)
    hypN_all = hyp_b.astype(f8)
    hypT_all = np.ascontiguousarray(hyp_b.transpose(0, 2, 1)).astype(f8)

    # fused scoring weights: WS[h*16+q, n] = sum_k W[q,k] Wmh[h,k,n]
    WS = np.einsum('qk,hkn->hqn', W, Wmh).reshape(128, N)
    bSp = (np.einsum('qk,hk->hq', W, bmh).reshape(128)
           + np.tile(bW, H)).astype(np.float32)
    WSm = np.einsum('qk,hkn->hqn', Wm, Wmh).reshape(128, N)
    bSm = (np.einsum('qk,hk->hq', Wm, bmh).reshape(128)
           + np.tile(bWm, H)).astype(np.float32)

    # per-batch time-mean and everything that depends only on it (host f32)
    xbar = np.asarray(hyp_b.mean(axis=1, dtype=np.float64), np.float32)
    sbias_all = (xbar @ WS.T + bSp).astype(np.float32)            # (B, 128)
    mw_all = np.tanh(xbar @ WSm.T + bSm).astype(np.float32)       # (B, 128)
    mbar_all = (np.einsum('bn,hkn->bhk', xbar, Wmh)
                + bmh).astype(np.float32)                         # (B, H, K)

    # WST (128, NCH, 128): [p, c, q] = WS_SCALE * WS[q, c*128+p]
    WST = np.ascontiguousarray(
        (WS * WS_SCALE).T.reshape(NCH, 128, 128).transpose(1, 0, 2)).astype(f8)
    # Wmh (H, K, N) -> (128, H, NCH, K): [p, h, c, k] = WMH_SCALE*Wmh[h,k,c*128+p]
    wmhT = np.ascontiguousarray(
        (Wmh * WMH_SCALE).transpose(2, 0, 1).reshape(NCH, 128, H, K)
        .transpose(1, 2, 0, 3)).astype(f8)
    whD = np.zeros((K, H), dtype=np.float32)
    for h in range(H):
        whD[h * K2:(h + 1) * K2, h] = Wh
    whD = whD.astype(bf)

    in_maps = []
    for c in range(NCORES):
        sl = slice(c * BL, (c + 1) * BL)
        aux = np.empty((128, 2 * BL + BL * H), np.float32)
        aux[:, :BL] = sbias_all[sl].T
        aux[:, BL:2 * BL] = mw_all[sl].T
        # mbarT columns: [2BL + bl*H + h] = mbar[bl, h, :]
        aux[:, 2 * BL:] = mbar_all[sl].reshape(BL * H, K).T
        in_maps.append({
            "hypT": np.ascontiguousarray(hypT_all[sl]),
            "hypN": np.ascontiguousarray(hypN_all[sl]),
            "WST": WST, "whD": whD, "wmhT": wmhT,
            "aux": aux,
        })
    return in_maps


def kernel(hyp, Wmh, bmh, W, bW, Wm, bWm, Wh, bWh,
           dan_hidden_size=None, attention_hidden_size=None,
           multihead_size=None, **_):
    from concourse.bass_utils import run_bass_kernel_spmd

    in_maps = _prep_inputs(hyp, Wmh, bmh, W, bW, Wm, bWm, Wh, bWh)
    if "nc" not in _cache:
        _cache["nc"] = _build_nc()
    res = run_bass_kernel_spmd(_cache["nc"], in_maps, core_ids=list(range(NCORES)))
    # out is (K, BL, H) per core -> (BL, H, K) -> (BL, N)
    out = np.concatenate([r["out"].transpose(1, 2, 0).reshape(BL, N)
                          for r in res.results], axis=0)
    return out.astype(np.float32)


# revision 39
# speedup vs baseline: 1.0187x; 1.0187x over previous
"""Trainium2 Bass kernel for nn_Attention_46454366273781 (sparse_attention).

Reference computation (T=2048, B=32, N=1024, H=8, K=128, K2=16):
    X = einsum('tbn,hkn->bthk', hyp, Wmh) + bmh          # per-head projections
    m = X.mean(axis=1)                                   # mean over time
    g = tanh(X @ W.T + bW) * tanh(m @ Wm.T + bWm)[:,None]
    s = g @ Wh + bWh ; a = softmax(s, axis=time)
    c = einsum('bth,bthk->bhk', a, X) ; out = c.reshape(B, H*K)

Key algebra: X itself is never needed on device.
  * scoring:  X @ W.T + bW  =  hyp @ WS.T + sbias  with WS = W @ Wmh (per head)
    and sbias = bSp + WS @ mean_t(hyp)  (scoring split around the time-mean)
  * gate:     tanh(m @ Wm.T + bWm) = tanh(WSm @ mean_t(hyp) + bSm) -- depends
    only on the time-mean, so it is a tiny per-batch vector.
  * output:   softmax weights sum to 1, so with x_bar = mean_t(hyp):
        c = [x_bar + sum_t (a_t - 1/T) hyp_t] @ Wmh^T + bmh
    The x_bar part (plus bias) is exact; the device only computes the small
    deviation term with zero-sum weights w = S*(p/Z - 1/T), which kills the
    systematic component of fp8 rounding noise.

hyp is streamed in fp8e4 in BOTH layouts (N-major for the rank-128 scoring
matmul, T-major for the deviation-weighted time reduction), with DoubleRow
fp8 matmuls (2 contraction rows/partition).  Per-core HBM traffic ~17MB,
making the kernel DMA-bound at ~360 GB/s.  Sharding: data-parallel over
batch B across 8 cores (4 batches/core).  bWh cancels inside the softmax.
"""

import numpy as np
import ml_dtypes

T, B, N, H = 2048, 32, 1024, 8
K, K2 = 128, 16          # per-head dim, attention hidden per head
NCORES = 8
BL = B // NCORES         # batches per core
NCH = N // 128           # contraction chunks over N
T128 = T // 128          # 128-sized time chunks
TC = 512                 # time chunk for scoring matmul free dim
TCH = T // TC            # time chunks (scoring)
S = 65536.0              # deviation-weight scale (keeps w in fp8 normal range)
HNU = 6                  # hN u-chunks loaded by DMA; the rest are produced
                         # on-chip by PE-transposing the resident hT tiles
WS_SCALE = 32.0          # WST prescale: WS values (std ~0.01) sit below fp8's
WMH_SCALE = 64.0         # min normal 2^-6; HW flushes fp8 subnormals to zero,
V_SCALE = 1.0 / 16.0     # so every fp8 tensor is kept in the normal range via
                         # power-of-2 prescales that are divided back out.

_cache = {}


def _build_nc():
    import concourse.mybir as mybir
    import concourse.tile as tile
    from concourse import bacc
    from concourse.masks import make_identity

    f8 = mybir.dt.float8e4
    bf16 = mybir.dt.bfloat16
    f32 = mybir.dt.float32
    AF = mybir.ActivationFunctionType
    AX = mybir.AxisListType
    OP = mybir.AluOpType
    DR = mybir.MatmulPerfMode.DoubleRow

    nc = bacc.Bacc("TRN2")
    hypT_d = nc.dram_tensor("hypT", (BL, N, T), f8, kind="ExternalInput")
    hypN_d = nc.dram_tensor("hypN", (BL, T, N), f8, kind="ExternalInput")
    WST_d = nc.dram_tensor("WST", (128, NCH, 128), f8, kind="ExternalInput")
    whD_d = nc.dram_tensor("whD", (K, H), bf16, kind="ExternalInput")
    wmhT_d = nc.dram_tensor("wmhT", (128, H, NCH, K), f8, kind="ExternalInput")
    # aux packs sbias [0:BL], mw [BL:2BL], mbarT [2BL + bl*H + h] per column
    aux_d = nc.dram_tensor("aux", (128, 2 * BL + BL * H), f32,
                           kind="ExternalInput")
    out_d = nc.dram_tensor("out", (K, BL, H), f32, kind="ExternalOutput")

    with tile.TileContext(nc) as tc, \
         tc.tile_pool(name="wpool", bufs=1) as wpool, \
         tc.tile_pool(name="hTp", bufs=3) as hTp, \
         tc.tile_pool(name="hNp", bufs=4) as hNp, \
         tc.tile_pool(name="g1p", bufs=3) as g1p, \
         tc.tile_pool(name="g2p", bufs=3) as g2p, \
         tc.tile_pool(name="seqp", bufs=2) as seqp, \
         tc.tile_pool(name="smallp", bufs=2) as smallp, \
         tc.tile_pool(name="psA", bufs=2, space="PSUM") as psA, \
         tc.tile_pool(name="psS", bufs=1, space="PSUM") as psS, \
         tc.tile_pool(name="psTp", bufs=2, space="PSUM") as psTp, \
         tc.tile_pool(name="psWTp", bufs=1, space="PSUM") as psWTp, \
         tc.tile_pool(name="psVp", bufs=1, space="PSUM") as psVp, \
         tc.tile_pool(name="psCp", bufs=1, space="PSUM") as psCp:

        # All loads go on the sync/HWDGE queue in explicit program order.
        # The scoring inputs (hT) are front-loaded so the last batch's long
        # scoring->softmax chain overlaps earlier DMA; only its hN (consumed
        # by the short v-pass) arrives last.  Stream order:
        #   hT0, aux, whD, WST, hT1, hN0, hT2, wmhT, hT3, hN1, hN2, hN3
        WST = wpool.tile([128, NCH, 128], f8)
        whD = wpool.tile([K, H], bf16)
        aux_sb = wpool.tile([128, 2 * BL + BL * H], f32)
        wmhT = wpool.tile([128, H, NCH, K], f8)
        ident = wpool.tile([128, 128], bf16)
        make_identity(nc, ident)
        identf8 = wpool.tile([128, 128], f8)
        make_identity(nc, identf8)
        c2all = wpool.tile([K, BL, H], f32)

        hTs = [hTp.tile([128, NCH, T], f8, tag="hT", name=f"hT_{i}")
               for i in range(BL)]
        hNs = [hNp.tile([128, T128, N], f8, tag="hN", name=f"hN_{i}")
               for i in range(BL)]
        TH = T // 2

        def load_hT(i):
            nc.sync.dma_start(
                out=hTs[i], in_=hypT_d[i].rearrange("(c p) t -> p c t", p=128))

        def load_hN(i):
            # only the first HNU time-chunks come from HBM; u >= HNU are
            # transposed on-chip from the hT tiles
            nc.sync.dma_start(
                out=hNs[i][:, :HNU, :],
                in_=hypN_d[i, :HNU * 128, :].rearrange("(u p) n -> p u n",
                                                       p=128))

        load_hT(0)
        nc.sync.dma_start(out=aux_sb, in_=aux_d[:])
        nc.sync.dma_start(out=whD, in_=whD_d[:])
        nc.sync.dma_start(out=WST, in_=WST_d[:])
        load_hT(1)
        load_hN(0)
        load_hT(2)
        load_hN(1)
        nc.sync.dma_start(out=wmhT, in_=wmhT_d[:])
        load_hT(3)
        load_hN(2)
        load_hN(3)

        wt8Ts = [None] * BL

        def phase_a(bl):
            """scoring + hN transposes + softmax + wt transpose for batch bl"""
            hT = hTs[bl]
            hN = hNs[bl]

            # ---- scoring: s = whD^T (tanh(WS hyp^T + sbias) * mw) ----
            s_exp = seqp.tile([8, T], f32, tag="s_exp", name=f"s_exp_{bl}")
            ssum_parts = smallp.tile([8, TCH], f32, tag="ssp", name=f"ssp_{bl}")
            for tci in range(TCH):
                tsl = slice(tci * TC, (tci + 1) * TC)
                ps = psA.tile([128, TC], f32, tag="psA", name=f"psA_{bl}_{tci}")
                for c in range(NCH // 2):
                    nc.tensor.matmul(ps, lhsT=WST[:, 2 * c:2 * c + 2, :],
                                     rhs=hT[:, 2 * c:2 * c + 2, tsl],
                                     start=(c == 0), stop=(c == NCH // 2 - 1),
                                     perf_mode=DR)
                g1 = g1p.tile([128, TC], bf16, tag="g1", name=f"g1_{bl}_{tci}")
                nc.scalar.activation(out=g1, in_=ps, func=AF.Tanh,
                                     scale=1.0 / WS_SCALE,
                                     bias=aux_sb[:, bl:bl + 1])
                g2 = g2p.tile([128, TC], bf16, tag="g2", name=f"g2_{bl}_{tci}")
                nc.vector.tensor_scalar_mul(g2, g1, aux_sb[:, BL + bl:BL + bl + 1])
                ps_s = psS.tile([8, TC], f32, tag="psS", name=f"psS_{bl}_{tci}")
                nc.tensor.matmul(ps_s, lhsT=whD, rhs=g2, start=True, stop=True)
                nc.scalar.activation(out=s_exp[:, tsl], in_=ps_s, func=AF.Exp,
                                     accum_out=ssum_parts[:, tci:tci + 1])

            # ---- zero-sum deviation weights w = S*(p/Z - 1/T) ----
            # ---- build hN u-chunks >= HNU by transposing hT on the PE;
            # ---- fills the PE while the softmax chain drains ----
            for ub in range(HNU // 2, T128 // 2):
                psT = psTp.tile([128, 2, N], f8, tag="psT",
                                name=f"psT_{bl}_{ub}")
                for j in range(2):
                    u = 2 * ub + j
                    for c in range(NCH):
                        nc.tensor.matmul(
                            psT[:, j, c * 128:(c + 1) * 128],
                            lhsT=hT[:, c, u * 128:(u + 1) * 128],
                            rhs=identf8, is_transpose=True,
                            skip_group_check=True)
                eng = (nc.vector, nc.gpsimd, nc.vector, nc.gpsimd,
                       nc.gpsimd)[ub - HNU // 2]
                eng.tensor_copy(out=hN[:, 2 * ub:2 * ub + 2, :], in_=psT)

            # wt = (p - Z/T) * (S/Z): the subtraction uses Z exactly so the
            # weights stay zero-sum even though the HW reciprocal is
            # approximate -- a reciprocal error then only scales the small
            # deviation term instead of leaking the full mean into c.
            ssum = smallp.tile([8, 1], f32, tag="ssum", name=f"ssum_{bl}")
            nc.vector.reduce_sum(out=ssum, in_=ssum_parts, axis=AX.X)
            zs = smallp.tile([8, 1], f32, tag="zs", name=f"zs_{bl}")
            nc.vector.tensor_scalar_mul(zs, ssum, 1.0 / S)
            sinvS = smallp.tile([8, 1], f32, tag="sinvS", name=f"sinvS_{bl}")
            nc.vector.reciprocal(sinvS, zs)
            zT = smallp.tile([8, 1], f32, tag="zT", name=f"zT_{bl}")
            nc.vector.tensor_scalar_mul(zT, ssum, 1.0 / T)
            negb = smallp.tile([8, 1], f32, tag="negb", name=f"negb_{bl}")
            nc.vector.tensor_scalar(out=negb, in0=zT, scalar1=sinvS,
                                    scalar2=-1.0, op0=OP.mult, op1=OP.mult)
            wt = seqp.tile([8, T], bf16, tag="wt", name=f"wt_{bl}")
            # split across ACT and DVE: this sits on the tail critical path
            nc.scalar.activation(out=wt[:, :TH], in_=s_exp[:, :TH],
                                 func=AF.Identity, scale=sinvS, bias=negb)
            nc.vector.tensor_scalar(out=wt[:, TH:], in0=s_exp[:, TH:],
                                    scalar1=zT, scalar2=sinvS, op0=OP.subtract,
                                    op1=OP.mult)

            # ---- transpose w to [t, h] and cast fp8 ----
            psWT = psWTp.tile([128, T128, 8], bf16, tag="psWT", name=f"psWT_{bl}")
            for u in range(T128):
                nc.tensor.matmul(psWT[:, u, :],
                                 lhsT=wt[:, u * 128:(u + 1) * 128],
                                 rhs=ident[:8, :8], is_transpose=True,
                                 skip_group_check=True)
            wt8T = smallp.tile([128, T128, 8], f8, tag="wt8T", name=f"wt8T_{bl}")
            nc.vector.tensor_copy(out=wt8T, in_=psWT)
            wt8Ts[bl] = wt8T

        def phase_b(bl):
            """deviation v-pass + output projection for batch bl"""
            hN = hNs[bl]
            wt8T = wt8Ts[bl]

            # ---- deviation v^T[n, h] = sum_t hyp[t, n] w[t, h] ----
            # n outer: each psV accumulation group must fully complete before
            # the next group's start=True, which pending-zeroes the whole
            # 2KB PSUM region and would wipe other groups' partial sums.
            psV = psVp.tile([128, NCH, 8], f32, tag="psV", name=f"psV_{bl}")
            for n in range(NCH):
                nsl = slice(n * 128, (n + 1) * 128)
                for u in range(T128 // 2):
                    nc.tensor.matmul(psV[:, n, :],
                                     lhsT=hN[:, 2 * u:2 * u + 2, nsl],
                                     rhs=wt8T[:, 2 * u:2 * u + 2, :],
                                     start=(u == 0), stop=(u == T128 // 2 - 1),
                                     perf_mode=DR, skip_group_check=True)
            vT8 = smallp.tile([128, NCH, 8], f8, tag="vT8", name=f"vT8_{bl}")
            nc.scalar.activation(out=vT8[:, :NCH // 2, :],
                                 in_=psV[:, :NCH // 2, :], func=AF.Copy,
                                 scale=V_SCALE)
            nc.scalar.activation(out=vT8[:, NCH // 2:, :],
                                 in_=psV[:, NCH // 2:, :], func=AF.Copy,
                                 scale=V_SCALE)

            # ---- c_dev^T = Wmh v_dev ; c = c_dev/S + mbar ----
            # h outer for the same reason: complete each column's group first
            ps_c = psCp.tile([128, H], f32, tag="ps_c", name=f"ps_c_{bl}")
            for h in range(H):
                for n in range(NCH // 2):
                    nc.tensor.matmul(ps_c[:, h:h + 1],
                                     lhsT=wmhT[:, h, 2 * n:2 * n + 2, :],
                                     rhs=vT8[:, 2 * n:2 * n + 2, h:h + 1],
                                     start=(n == 0), stop=(n == NCH // 2 - 1),
                                     perf_mode=DR, skip_group_check=True)
            nc.vector.scalar_tensor_tensor(
                out=c2all[:, bl, :], in0=ps_c,
                scalar=1.0 / (S * WMH_SCALE * V_SCALE),
                in1=aux_sb[:, 2 * BL + bl * H:2 * BL + (bl + 1) * H],
                op0=OP.mult, op1=OP.add)

        # software pipeline: phase_b(bl) is emitted one batch behind so the
        # in-order PE queue never stalls on the transpose copies of batch bl
        for bl in range(BL):
            phase_a(bl)
            if bl >= 1:
                phase_b(bl - 1)
        phase_b(BL - 1)

        nc.sync.dma_start(out=out_d[:], in_=c2all)

    nc.compile()
    return nc


def _prep_inputs(hyp, Wmh, bmh, W, bW, Wm, bWm, Wh, bWh):
    """Host-side sharding + layout prep (numpy only)."""
    f8 = ml_dtypes.float8_e4m3
    bf = ml_dtypes.bfloat16
    hyp = np.asarray(hyp, np.float32)
    Wmh = np.asarray(Wmh, np.float32)
    bmh = np.asarray(bmh, np.float32)
    W = np.asarray(W, np.float32)
    bW = np.asarray(bW, np.float32)
    Wm = np.asarray(Wm, np.float32)
    bWm = np.asarray(bWm, np.float32)
    Wh = np.asarray(Wh, np.float32)

    hyp_b = np.ascontiguousarray(hyp.transpose(1, 0, 2))          # (B, T, N)
    hypN_all = hyp_b.astype(f8)
    hypT_all = np.ascontiguousarray(hyp_b.transpose(0, 2, 1)).astype(f8)

    # fused scoring weights: WS[h*16+q, n] = sum_k W[q,k] Wmh[h,k,n]
    WS = np.einsum('qk,hkn->hqn', W, Wmh).reshape(128, N)
    bSp = (np.einsum('qk,hk->hq', W, bmh).reshape(128)
           + np.tile(bW, H)).astype(np.float32)
    WSm = np.einsum('qk,hkn->hqn', Wm, Wmh).reshape(128, N)
    bSm = (np.einsum('qk,hk->hq', Wm, bmh).reshape(128)
           + np.tile(bWm, H)).astype(np.float32)

    # per-batch time-mean and everything that depends only on it (host f32)
    xbar = np.asarray(hyp_b.mean(axis=1, dtype=np.float64), np.float32)
    sbias_all = (xbar @ WS.T + bSp).astype(np.float32)            # (B, 128)
    mw_all = np.tanh(xbar @ WSm.T + bSm).astype(np.float32)       # (B, 128)
    mbar_all = (np.einsum('bn,hkn->bhk', xbar, Wmh)
                + bmh).astype(np.float32)                         # (B, H, K)

    # WST (128, NCH, 128): [p, c, q] = WS_SCALE * WS[q, c*128+p]
    WST = np.ascontiguousarray(
        (WS * WS_SCALE).T.reshape(NCH, 128, 128).transpose(1, 0, 2)).astype(f8)
    # Wmh (H, K, N) -> (128, H, NCH, K): [p, h, c, k] = WMH_SCALE*Wmh[h,k,c*128+p]
    wmhT = np.ascontiguousarray(
        (Wmh * WMH_SCALE).transpose(2, 0, 1).reshape(NCH, 128, H, K)
        .transpose(1, 2, 0, 3)).astype(f8)
    whD = np.zeros((K, H), dtype=np.float32)
    for h in range(H):
        whD[h * K2:(h + 1) * K2, h] = Wh
    whD = whD.astype(bf)

    in_maps = []
    for c in range(NCORES):
        sl = slice(c * BL, (c + 1) * BL)
        aux = np.empty((128, 2 * BL + BL * H), np.float32)
        aux[:, :BL] = sbias_all[sl].T
        aux[:, BL:2 * BL] = mw_all[sl].T
        # mbarT columns: [2BL + bl*H + h] = mbar[bl, h, :]
        aux[:, 2 * BL:] = mbar_all[sl].reshape(BL * H, K).T
        in_maps.append({
            "hypT": np.ascontiguousarray(hypT_all[sl]),
            "hypN": np.ascontiguousarray(hypN_all[sl]),
            "WST": WST, "whD": whD, "wmhT": wmhT,
            "aux": aux,
        })
    return in_maps


def kernel(hyp, Wmh, bmh, W, bW, Wm, bWm, Wh, bWh,
           dan_hidden_size=None, attention_hidden_size=None,
           multihead_size=None, **_):
    from concourse.bass_utils import run_bass_kernel_spmd

    in_maps = _prep_inputs(hyp, Wmh, bmh, W, bW, Wm, bWm, Wh, bWh)
    if "nc" not in _cache:
        _cache["nc"] = _build_nc()
    res = run_bass_kernel_spmd(_cache["nc"], in_maps, core_ids=list(range(NCORES)))
    # out is (K, BL, H) per core -> (BL, H, K) -> (BL, N)
    out = np.concatenate([r["out"].transpose(1, 2, 0).reshape(BL, N)
                          for r in res.results], axis=0)
    return out.astype(np.float32)


# revision 42
# speedup vs baseline: 1.1724x; 1.1509x over previous
"""Trainium2 Bass kernel for nn_Attention_46454366273781 (sparse_attention).

Reference computation (T=2048, B=32, N=1024, H=8, K=128, K2=16):
    X = einsum('tbn,hkn->bthk', hyp, Wmh) + bmh          # per-head projections
    m = X.mean(axis=1)                                   # mean over time
    g = tanh(X @ W.T + bW) * tanh(m @ Wm.T + bWm)[:,None]
    s = g @ Wh + bWh ; a = softmax(s, axis=time)
    c = einsum('bth,bthk->bhk', a, X) ; out = c.reshape(B, H*K)

Key algebra: X itself is never needed on device.
  * scoring:  X @ W.T + bW  =  hyp @ WS.T + sbias  with WS = W @ Wmh (per head)
    and sbias = bSp + WS @ mean_t(hyp)  (scoring split around the time-mean)
  * gate:     tanh(m @ Wm.T + bWm) = tanh(WSm @ mean_t(hyp) + bSm) -- depends
    only on the time-mean, so it is a tiny per-batch vector.
  * output:   softmax weights sum to 1, so with x_bar = mean_t(hyp):
        c = [x_bar + sum_t (a_t - 1/T) hyp_t] @ Wmh^T + bmh
    The x_bar part (plus bias) is exact; the device only computes the small
    deviation term with zero-sum weights w = S*(p/Z - 1/T), which kills the
    systematic component of fp8 rounding noise.

hyp is streamed in fp8e4 in BOTH layouts (N-major for the rank-128 scoring
matmul, T-major for the deviation-weighted time reduction), with DoubleRow
fp8 matmuls (2 contraction rows/partition).  Per-core HBM traffic ~17MB,
making the kernel DMA-bound at ~360 GB/s.  Sharding: data-parallel over
batch B across 8 cores (4 batches/core).  bWh cancels inside the softmax.
"""

import numpy as np
import ml_dtypes

T, B, N, H = 2048, 32, 1024, 8
K, K2 = 128, 16          # per-head dim, attention hidden per head
NCORES = 8
BL = B // NCORES         # batches per core
NCH = N // 128           # contraction chunks over N
T128 = T // 128          # 128-sized time chunks
TC = 512                 # time chunk for scoring matmul free dim
TCH = T // TC            # time chunks (scoring)
S = 65536.0              # deviation-weight scale (keeps w in fp8 normal range)
HNU = 8                  # hN u-chunks loaded by DMA; the rest are produced
                         # on-chip by PE-transposing the resident hT tiles
WS_SCALE = 32.0          # WST prescale: WS values (std ~0.01) sit below fp8's
WMH_SCALE = 64.0         # min normal 2^-6; HW flushes fp8 subnormals to zero,
V_SCALE = 1.0 / 16.0     # so every fp8 tensor is kept in the normal range via
                         # power-of-2 prescales that are divided back out.

_cache = {}


def _build_nc():
    import concourse.mybir as mybir
    import concourse.tile as tile
    from concourse import bacc
    from concourse.masks import make_identity

    f8 = mybir.dt.float8e4
    bf16 = mybir.dt.bfloat16
    f32 = mybir.dt.float32
    AF = mybir.ActivationFunctionType
    AX = mybir.AxisListType
    OP = mybir.AluOpType
    DR = mybir.MatmulPerfMode.DoubleRow

    nc = bacc.Bacc("TRN2")
    hypT_d = nc.dram_tensor("hypT", (BL, N, T), f8, kind="ExternalInput")
    hypN_d = nc.dram_tensor("hypN", (BL, T, N), f8, kind="ExternalInput")
    WST_d = nc.dram_tensor("WST", (128, NCH, 128), f8, kind="ExternalInput")
    whD_d = nc.dram_tensor("whD", (K, H), bf16, kind="ExternalInput")
    wmhT_d = nc.dram_tensor("wmhT", (128, H, NCH, K), f8, kind="ExternalInput")
    # aux packs sbias [0:BL], mw [BL:2BL], mbarT [2BL + bl*H + h] per column
    aux_d = nc.dram_tensor("aux", (128, 2 * BL + BL * H), f32,
                           kind="ExternalInput")
    out_d = nc.dram_tensor("out", (K, BL, H), f32, kind="ExternalOutput")

    with tile.TileContext(nc) as tc, \
         tc.tile_pool(name="wpool", bufs=1) as wpool, \
         tc.tile_pool(name="hTp", bufs=3) as hTp, \
         tc.tile_pool(name="hNp", bufs=4) as hNp, \
         tc.tile_pool(name="g1p", bufs=3) as g1p, \
         tc.tile_pool(name="g2p", bufs=3) as g2p, \
         tc.tile_pool(name="seqp", bufs=2) as seqp, \
         tc.tile_pool(name="smallp", bufs=2) as smallp, \
         tc.tile_pool(name="psA", bufs=2, space="PSUM") as psA, \
         tc.tile_pool(name="psS", bufs=1, space="PSUM") as psS, \
         tc.tile_pool(name="psTp", bufs=2, space="PSUM") as psTp, \
         tc.tile_pool(name="psWTp", bufs=1, space="PSUM") as psWTp, \
         tc.tile_pool(name="psVp", bufs=1, space="PSUM") as psVp, \
         tc.tile_pool(name="psCp", bufs=1, space="PSUM") as psCp:

        # All loads go on the sync/HWDGE queue in explicit program order.
        # The scoring inputs (hT) are front-loaded so the last batch's long
        # scoring->softmax chain overlaps earlier DMA; only its hN (consumed
        # by the short v-pass) arrives last.  Stream order:
        #   hT0, aux, whD, WST, hT1, hN0, hT2, wmhT, hT3, hN1, hN2, hN3
        WST = wpool.tile([128, NCH, 128], f8)
        whD = wpool.tile([K, H], bf16)
        aux_sb = wpool.tile([128, 2 * BL + BL * H], f32)
        wmhT = wpool.tile([128, H, NCH, K], f8)
        ident = wpool.tile([128, 128], bf16)
        make_identity(nc, ident)
        identf8 = wpool.tile([128, 128], f8)
        make_identity(nc, identf8)
        c2all = wpool.tile([K, BL, H], f32)

        hTs = [hTp.tile([128, NCH, T], f8, tag="hT", name=f"hT_{i}")
               for i in range(BL)]
        hNs = [hNp.tile([128, T128, N], f8, tag="hN", name=f"hN_{i}")
               for i in range(BL)]
        TH = T // 2

        def load_hT(i):
            nc.sync.dma_start(
                out=hTs[i], in_=hypT_d[i].rearrange("(c p) t -> p c t", p=128))

        def load_hN(i):
            # only the first HNU time-chunks come from HBM; u >= HNU are
            # transposed on-chip from the hT tiles
            nc.sync.dma_start(
                out=hNs[i][:, :HNU, :],
                in_=hypN_d[i, :HNU * 128, :].rearrange("(u p) n -> p u n",
                                                       p=128))

        load_hT(0)
        nc.sync.dma_start(out=aux_sb, in_=aux_d[:])
        nc.sync.dma_start(out=whD, in_=whD_d[:])
        nc.sync.dma_start(out=WST, in_=WST_d[:])
        load_hT(1)
        load_hN(0)
        load_hT(2)
        load_hN(1)
        nc.sync.dma_start(out=wmhT, in_=wmhT_d[:])
        load_hT(3)
        load_hN(2)
        load_hN(3)

        wt8Ts = [None] * BL

        def phase_a(bl):
            """scoring + hN transposes + softmax + wt transpose for batch bl"""
            hT = hTs[bl]
            hN = hNs[bl]

            # ---- scoring: s = whD^T (tanh(WS hyp^T + sbias) * mw) ----
            s_exp = seqp.tile([8, T], f32, tag="s_exp", name=f"s_exp_{bl}")
            ssum_parts = smallp.tile([8, TCH], f32, tag="ssp", name=f"ssp_{bl}")
            for tci in range(TCH):
                tsl = slice(tci * TC, (tci + 1) * TC)
                ps = psA.tile([128, TC], f32, tag="psA", name=f"psA_{bl}_{tci}")
                for c in range(NCH // 2):
                    nc.tensor.matmul(ps, lhsT=WST[:, 2 * c:2 * c + 2, :],
                                     rhs=hT[:, 2 * c:2 * c + 2, tsl],
                                     start=(c == 0), stop=(c == NCH // 2 - 1),
                                     perf_mode=DR)
                g1 = g1p.tile([128, TC], bf16, tag="g1", name=f"g1_{bl}_{tci}")
                nc.scalar.activation(out=g1, in_=ps, func=AF.Tanh,
                                     scale=1.0 / WS_SCALE,
                                     bias=aux_sb[:, bl:bl + 1])
                g2 = g2p.tile([128, TC], bf16, tag="g2", name=f"g2_{bl}_{tci}")
                nc.vector.tensor_scalar_mul(g2, g1, aux_sb[:, BL + bl:BL + bl + 1])
                ps_s = psS.tile([8, TC], f32, tag="psS", name=f"psS_{bl}_{tci}")
                nc.tensor.matmul(ps_s, lhsT=whD, rhs=g2, start=True, stop=True)
                nc.scalar.activation(out=s_exp[:, tsl], in_=ps_s, func=AF.Exp,
                                     accum_out=ssum_parts[:, tci:tci + 1])

            # ---- zero-sum deviation weights w = S*(p/Z - 1/T) ----
            # ---- build hN u-chunks >= HNU by transposing hT on the PE;
            # ---- fills the PE while the softmax chain drains ----
            for ub in range(HNU // 2, T128 // 2):
                psT = psTp.tile([128, 2, N], f8, tag="psT",
                                name=f"psT_{bl}_{ub}")
                for j in range(2):
                    u = 2 * ub + j
                    for c in range(NCH):
                        nc.tensor.matmul(
                            psT[:, j, c * 128:(c + 1) * 128],
                            lhsT=hT[:, c, u * 128:(u + 1) * 128],
                            rhs=identf8, is_transpose=True,
                            skip_group_check=True)
                eng = (nc.vector, nc.gpsimd, nc.vector, nc.gpsimd,
                       nc.vector, nc.gpsimd)[(ub - HNU // 2) % 6]
                eng.tensor_copy(out=hN[:, 2 * ub:2 * ub + 2, :], in_=psT)

            # wt = (p - Z/T) * (S/Z): the subtraction uses Z exactly so the
            # weights stay zero-sum even though the HW reciprocal is
            # approximate -- a reciprocal error then only scales the small
            # deviation term instead of leaking the full mean into c.
            ssum = smallp.tile([8, 1], f32, tag="ssum", name=f"ssum_{bl}")
            nc.vector.reduce_sum(out=ssum, in_=ssum_parts, axis=AX.X)
            zs = smallp.tile([8, 1], f32, tag="zs", name=f"zs_{bl}")
            nc.vector.tensor_scalar_mul(zs, ssum, 1.0 / S)
            sinvS = smallp.tile([8, 1], f32, tag="sinvS", name=f"sinvS_{bl}")
            nc.vector.reciprocal(sinvS, zs)
            zT = smallp.tile([8, 1], f32, tag="zT", name=f"zT_{bl}")
            nc.vector.tensor_scalar_mul(zT, ssum, 1.0 / T)
            negb = smallp.tile([8, 1], f32, tag="negb", name=f"negb_{bl}")
            nc.vector.tensor_scalar(out=negb, in0=zT, scalar1=sinvS,
                                    scalar2=-1.0, op0=OP.mult, op1=OP.mult)
            wt = seqp.tile([8, T], bf16, tag="wt", name=f"wt_{bl}")
            # split across ACT and DVE: this sits on the tail critical path
            nc.scalar.activation(out=wt[:, :TH], in_=s_exp[:, :TH],
                                 func=AF.Identity, scale=sinvS, bias=negb)
            nc.vector.tensor_scalar(out=wt[:, TH:], in0=s_exp[:, TH:],
                                    scalar1=zT, scalar2=sinvS, op0=OP.subtract,
                                    op1=OP.mult)

            wt8Ts[bl] = wt

        def phase_b(bl):
            """wt transpose + deviation v-pass + output projection, batch bl"""
            hN = hNs[bl]
            wt = wt8Ts[bl]

            # ---- transpose w to [t, h] and cast fp8; by now wt is long
            # ---- ready, so the PE never stalls on the softmax chain ----
            psWT = psWTp.tile([128, T128, 8], bf16, tag="psWT", name=f"psWT_{bl}")
            for u in range(T128):
                nc.tensor.matmul(psWT[:, u, :],
                                 lhsT=wt[:, u * 128:(u + 1) * 128],
                                 rhs=ident[:8, :8], is_transpose=True,
                                 skip_group_check=True)
            wt8T = smallp.tile([128, T128, 8], f8, tag="wt8T", name=f"wt8T_{bl}")
            nc.vector.tensor_copy(out=wt8T, in_=psWT)

            # ---- deviation v^T[n, h] = sum_t hyp[t, n] w[t, h] ----
            # n outer: each psV accumulation group must fully complete before
            # the next group's start=True, which pending-zeroes the whole
            # 2KB PSUM region and would wipe other groups' partial sums.
            psV = psVp.tile([128, NCH, 8], f32, tag="psV", name=f"psV_{bl}")
            for n in range(NCH):
                nsl = slice(n * 128, (n + 1) * 128)
                for u in range(T128 // 2):
                    nc.tensor.matmul(psV[:, n, :],
                                     lhsT=hN[:, 2 * u:2 * u + 2, nsl],
                                     rhs=wt8T[:, 2 * u:2 * u + 2, :],
                                     start=(u == 0), stop=(u == T128 // 2 - 1),
                                     perf_mode=DR, skip_group_check=True)
            vT8 = smallp.tile([128, NCH, 8], f8, tag="vT8", name=f"vT8_{bl}")
            nc.scalar.activation(out=vT8[:, :NCH // 2, :],
                                 in_=psV[:, :NCH // 2, :], func=AF.Copy,
                                 scale=V_SCALE)
            nc.scalar.activation(out=vT8[:, NCH // 2:, :],
                                 in_=psV[:, NCH // 2:, :], func=AF.Copy,
                                 scale=V_SCALE)

            # ---- c_dev^T = Wmh v_dev ; c = c_dev/S + mbar ----
            # h outer for the same reason: complete each column's group first
            ps_c = psCp.tile([128, H], f32, tag="ps_c", name=f"ps_c_{bl}")
            for h in range(H):
                for n in range(NCH // 2):
                    nc.tensor.matmul(ps_c[:, h:h + 1],
                                     lhsT=wmhT[:, h, 2 * n:2 * n + 2, :],
                                     rhs=vT8[:, 2 * n:2 * n + 2, h:h + 1],
                                     start=(n == 0), stop=(n == NCH // 2 - 1),
                                     perf_mode=DR, skip_group_check=True)
            nc.vector.scalar_tensor_tensor(
                out=c2all[:, bl, :], in0=ps_c,
                scalar=1.0 / (S * WMH_SCALE * V_SCALE),
                in1=aux_sb[:, 2 * BL + bl * H:2 * BL + (bl + 1) * H],
                op0=OP.mult, op1=OP.add)

        # software pipeline: phase_b(bl) is emitted one batch behind so the
        # in-order PE queue never stalls on the transpose copies of batch bl
        for bl in range(BL):
            phase_a(bl)
            if bl >= 1:
                phase_b(bl - 1)
        phase_b(BL - 1)

        nc.sync.dma_start(out=out_d[:], in_=c2all)

    nc.compile()
    return nc


def _prep_inputs(hyp, Wmh, bmh, W, bW, Wm, bWm, Wh, bWh):
    """Host-side sharding + layout prep (numpy only)."""
    f8 = ml_dtypes.float8_e4m3
    bf = ml_dtypes.bfloat16
    hyp = np.asarray(hyp, np.float32)
    Wmh = np.asarray(Wmh, np.float32)
    bmh = np.asarray(bmh, np.float32)
    W = np.asarray(W, np.float32)
    bW = np.asarray(bW, np.float32)
    Wm = np.asarray(Wm, np.float32)
    bWm = np.asarray(bWm, np.float32)
    Wh = np.asarray(Wh, np.float32)

    hyp_b = np.ascontiguousarray(hyp.transpose(1, 0, 2))          # (B, T, N)
    hypN_all = hyp_b.astype(f8)
    hypT_all = np.ascontiguousarray(hyp_b.transpose(0, 2, 1)).astype(f8)

    # fused scoring weights: WS[h*16+q, n] = sum_k W[q,k] Wmh[h,k,n]
    WS = np.einsum('qk,hkn->hqn', W, Wmh).reshape(128, N)
    bSp = (np.einsum('qk,hk->hq', W, bmh).reshape(128)
           + np.tile(bW, H)).astype(np.float32)
    WSm = np.einsum('qk,hkn->hqn', Wm, Wmh).reshape(128, N)
    bSm = (np.einsum('qk,hk->hq', Wm, bmh).reshape(128)
           + np.tile(bWm, H)).astype(np.float32)

    # per-batch time-mean and everything that depends only on it (host f32)
    xbar = np.asarray(hyp_b.mean(axis=1, dtype=np.float64), np.float32)
    sbias_all = (xbar @ WS.T + bSp).astype(np.float32)            # (B, 128)
    mw_all = np.tanh(xbar @ WSm.T + bSm).astype(np.float32)       # (B, 128)
    mbar_all = (np.einsum('bn,hkn->bhk', xbar, Wmh)
                + bmh).astype(np.float32)                         # (B, H, K)

    # WST (128, NCH, 128): [p, c, q] = WS_SCALE * WS[q, c*128+p]
    WST = np.ascontiguousarray(
        (WS * WS_SCALE).T.reshape(NCH, 128, 128).transpose(1, 0, 2)).astype(f8)
    # Wmh (H, K, N) -> (128, H, NCH, K): [p, h, c, k] = WMH_SCALE*Wmh[h,k,c*128+p]
    wmhT = np.ascontiguousarray(
        (Wmh * WMH_SCALE).transpose(2, 0, 1).reshape(NCH, 128, H, K)
        .transpose(1, 2, 0, 3)).astype(f8)
    whD = np.zeros((K, H), dtype=np.float32)
    for h in range(H):
        whD[h * K2:(h + 1) * K2, h] = Wh
    whD = whD.astype(bf)

    in_maps = []
    for c in range(NCORES):
        sl = slice(c * BL, (c + 1) * BL)
        aux = np.empty((128, 2 * BL + BL * H), np.float32)
        aux[:, :BL] = sbias_all[sl].T
        aux[:, BL:2 * BL] = mw_all[sl].T
        # mbarT columns: [2BL + bl*H + h] = mbar[bl, h, :]
        aux[:, 2 * BL:] = mbar_all[sl].reshape(BL * H, K).T
        in_maps.append({
            "hypT": np.ascontiguousarray(hypT_all[sl]),
            "hypN": np.ascontiguousarray(hypN_all[sl]),
            "WST": WST, "whD": whD, "wmhT": wmhT,
            "aux": aux,
        })
    return in_maps


def kernel(hyp, Wmh, bmh, W, bW, Wm, bWm, Wh, bWh,
           dan_hidden_size=None, attention_hidden_size=None,
           multihead_size=None, **_):
    from concourse.bass_utils import run_bass_kernel_spmd

    in_maps = _prep_inputs(hyp, Wmh, bmh, W, bW, Wm, bWm, Wh, bWh)
    if "nc" not in _cache:
        _cache["nc"] = _build_nc()
    res = run_bass_kernel_spmd(_cache["nc"], in_maps, core_ids=list(range(NCORES)))
    # out is (K, BL, H) per core -> (BL, H, K) -> (BL, N)
    out = np.concatenate([r["out"].transpose(1, 2, 0).reshape(BL, N)
                          for r in res.results], axis=0)
    return out.astype(np.float32)


# revision 44
# speedup vs baseline: 1.2016x; 1.0249x over previous
"""Trainium2 Bass kernel for nn_Attention_46454366273781 (sparse_attention).

Reference computation (T=2048, B=32, N=1024, H=8, K=128, K2=16):
    X = einsum('tbn,hkn->bthk', hyp, Wmh) + bmh          # per-head projections
    m = X.mean(axis=1)                                   # mean over time
    g = tanh(X @ W.T + bW) * tanh(m @ Wm.T + bWm)[:,None]
    s = g @ Wh + bWh ; a = softmax(s, axis=time)
    c = einsum('bth,bthk->bhk', a, X) ; out = c.reshape(B, H*K)

Key algebra: X itself is never needed on device.
  * scoring:  X @ W.T + bW  =  hyp @ WS.T + sbias  with WS = W @ Wmh (per head)
    and sbias = bSp + WS @ mean_t(hyp)  (scoring split around the time-mean)
  * gate:     tanh(m @ Wm.T + bWm) = tanh(WSm @ mean_t(hyp) + bSm) -- depends
    only on the time-mean, so it is a tiny per-batch vector.
  * output:   softmax weights sum to 1, so with x_bar = mean_t(hyp):
        c = [x_bar + sum_t (a_t - 1/T) hyp_t] @ Wmh^T + bmh
    The x_bar part (plus bias) is exact; the device only computes the small
    deviation term with zero-sum weights w = S*(p/Z - 1/T), which kills the
    systematic component of fp8 rounding noise.

hyp is streamed in fp8e4 in BOTH layouts (N-major for the rank-128 scoring
matmul, T-major for the deviation-weighted time reduction), with DoubleRow
fp8 matmuls (2 contraction rows/partition).  Per-core HBM traffic ~17MB,
making the kernel DMA-bound at ~360 GB/s.  Sharding: data-parallel over
batch B across 8 cores (4 batches/core).  bWh cancels inside the softmax.
"""

import numpy as np
import ml_dtypes

T, B, N, H = 2048, 32, 1024, 8
K, K2 = 128, 16          # per-head dim, attention hidden per head
NCORES = 8
BL = B // NCORES         # batches per core
NCH = N // 128           # contraction chunks over N
T128 = T // 128          # 128-sized time chunks
TC = 512                 # time chunk for scoring matmul free dim
TCH = T // TC            # time chunks (scoring)
S = 65536.0              # deviation-weight scale (keeps w in fp8 normal range)
HNU = 8                  # hN u-chunks loaded by DMA; the rest are produced
                         # on-chip by PE-transposing the resident hT tiles
WS_SCALE = 32.0          # WST prescale: WS values (std ~0.01) sit below fp8's
WMH_SCALE = 64.0         # min normal 2^-6; HW flushes fp8 subnormals to zero,
V_SCALE = 1.0 / 16.0     # so every fp8 tensor is kept in the normal range via
                         # power-of-2 prescales that are divided back out.

_cache = {}


def _build_nc():
    import concourse.mybir as mybir
    import concourse.tile as tile
    from concourse import bacc
    from concourse.masks import make_identity

    f8 = mybir.dt.float8e4
    bf16 = mybir.dt.bfloat16
    f32 = mybir.dt.float32
    AF = mybir.ActivationFunctionType
    AX = mybir.AxisListType
    OP = mybir.AluOpType
    DR = mybir.MatmulPerfMode.DoubleRow

    nc = bacc.Bacc("TRN2")
    hypT_d = nc.dram_tensor("hypT", (BL, N, T), f8, kind="ExternalInput")
    hypN_d = nc.dram_tensor("hypN", (BL, T, N), f8, kind="ExternalInput")
    WST_d = nc.dram_tensor("WST", (128, NCH, 128), f8, kind="ExternalInput")
    whD_d = nc.dram_tensor("whD", (K, H), bf16, kind="ExternalInput")
    wmhT_d = nc.dram_tensor("wmhT", (128, H, NCH, K), f8, kind="ExternalInput")
    # aux packs sbias [0:BL], mw [BL:2BL], mbarT [2BL + bl*H + h] per column
    aux_d = nc.dram_tensor("aux", (128, 2 * BL + BL * H), f32,
                           kind="ExternalInput")
    out_d = nc.dram_tensor("out", (K, BL, H), f32, kind="ExternalOutput")

    with tile.TileContext(nc) as tc, \
         tc.tile_pool(name="wpool", bufs=1) as wpool, \
         tc.tile_pool(name="hTp", bufs=4) as hTp, \
         tc.tile_pool(name="hNp", bufs=4) as hNp, \
         tc.tile_pool(name="g1p", bufs=3) as g1p, \
         tc.tile_pool(name="g2p", bufs=3) as g2p, \
         tc.tile_pool(name="seqp", bufs=2) as seqp, \
         tc.tile_pool(name="smallp", bufs=2) as smallp, \
         tc.tile_pool(name="psA", bufs=2, space="PSUM") as psA, \
         tc.tile_pool(name="psS", bufs=1, space="PSUM") as psS, \
         tc.tile_pool(name="psTp", bufs=2, space="PSUM") as psTp, \
         tc.tile_pool(name="psWTp", bufs=1, space="PSUM") as psWTp, \
         tc.tile_pool(name="psVp", bufs=1, space="PSUM") as psVp, \
         tc.tile_pool(name="psCp", bufs=1, space="PSUM") as psCp:

        # All loads go on the sync/HWDGE queue in explicit program order.
        # The scoring inputs (hT) are front-loaded so the last batch's long
        # scoring->softmax chain overlaps earlier DMA; only its hN (consumed
        # by the short v-pass) arrives last.  Stream order:
        #   hT0, aux, whD, WST, hT1, hN0, hT2, wmhT, hT3, hN1, hN2, hN3
        WST = wpool.tile([128, NCH, 128], f8)
        whD = wpool.tile([K, H], bf16)
        aux_sb = wpool.tile([128, 2 * BL + BL * H], f32)
        wmhT = wpool.tile([128, H, NCH, K], f8)
        ident = wpool.tile([128, 128], bf16)
        make_identity(nc, ident)
        identf8 = wpool.tile([128, 128], f8)
        make_identity(nc, identf8)
        c2all = wpool.tile([K, BL, H], f32)

        hTs = [hTp.tile([128, NCH, T], f8, tag="hT", name=f"hT_{i}")
               for i in range(BL)]
        hNs = [hNp.tile([128, T128, N], f8, tag="hN", name=f"hN_{i}")
               for i in range(BL)]
        TH = T // 2

        def load_hT(i):
            nc.sync.dma_start(
                out=hTs[i], in_=hypT_d[i].rearrange("(c p) t -> p c t", p=128))

        def load_hN(i):
            # only the first HNU time-chunks come from HBM; u >= HNU are
            # transposed on-chip from the hT tiles
            nc.sync.dma_start(
                out=hNs[i][:, :HNU, :],
                in_=hypN_d[i, :HNU * 128, :].rearrange("(u p) n -> p u n",
                                                       p=128))

        load_hT(0)
        nc.sync.dma_start(out=aux_sb, in_=aux_d[:])
        nc.sync.dma_start(out=whD, in_=whD_d[:])
        nc.sync.dma_start(out=WST, in_=WST_d[:])
        load_hT(1)
        load_hN(0)
        load_hT(2)
        load_hT(3)
        load_hN(1)
        nc.sync.dma_start(out=wmhT, in_=wmhT_d[:])
        load_hN(2)
        load_hN(3)

        wt8Ts = [None] * BL

        def phase_a(bl):
            """scoring + hN transposes + softmax + wt transpose for batch bl"""
            hT = hTs[bl]
            hN = hNs[bl]

            # ---- scoring: s = whD^T (tanh(WS hyp^T + sbias) * mw) ----
            s_exp = seqp.tile([8, T], f32, tag="s_exp", name=f"s_exp_{bl}")
            ssum_parts = smallp.tile([8, TCH], f32, tag="ssp", name=f"ssp_{bl}")
            for tci in range(TCH):
                tsl = slice(tci * TC, (tci + 1) * TC)
                ps = psA.tile([128, TC], f32, tag="psA", name=f"psA_{bl}_{tci}")
                for c in range(NCH // 2):
                    nc.tensor.matmul(ps, lhsT=WST[:, 2 * c:2 * c + 2, :],
                                     rhs=hT[:, 2 * c:2 * c + 2, tsl],
                                     start=(c == 0), stop=(c == NCH // 2 - 1),
                                     perf_mode=DR)
                g1 = g1p.tile([128, TC], bf16, tag="g1", name=f"g1_{bl}_{tci}")
                nc.scalar.activation(out=g1, in_=ps, func=AF.Tanh,
                                     scale=1.0 / WS_SCALE,
                                     bias=aux_sb[:, bl:bl + 1])
                g2 = g2p.tile([128, TC], bf16, tag="g2", name=f"g2_{bl}_{tci}")
                nc.vector.tensor_scalar_mul(g2, g1, aux_sb[:, BL + bl:BL + bl + 1])
                ps_s = psS.tile([8, TC], f32, tag="psS", name=f"psS_{bl}_{tci}")
                nc.tensor.matmul(ps_s, lhsT=whD, rhs=g2, start=True, stop=True)
                nc.scalar.activation(out=s_exp[:, tsl], in_=ps_s, func=AF.Exp,
                                     accum_out=ssum_parts[:, tci:tci + 1])

            # ---- zero-sum deviation weights w = S*(p/Z - 1/T) ----
            # ---- build hN u-chunks >= HNU by transposing hT on the PE;
            # ---- fills the PE while the softmax chain drains ----
            for ub in range(HNU // 2, T128 // 2):
                psT = psTp.tile([128, 2, N], f8, tag="psT",
                                name=f"psT_{bl}_{ub}")
                for j in range(2):
                    u = 2 * ub + j
                    for c in range(NCH):
                        nc.tensor.matmul(
                            psT[:, j, c * 128:(c + 1) * 128],
                            lhsT=hT[:, c, u * 128:(u + 1) * 128],
                            rhs=identf8, is_transpose=True,
                            skip_group_check=True)
                eng = (nc.vector, nc.gpsimd, nc.vector, nc.gpsimd,
                       nc.vector, nc.gpsimd)[(ub - HNU // 2) % 6]
                eng.tensor_copy(out=hN[:, 2 * ub:2 * ub + 2, :], in_=psT)

            # wt = (p - Z/T) * (S/Z): the subtraction uses Z exactly so the
            # weights stay zero-sum even though the HW reciprocal is
            # approximate -- a reciprocal error then only scales the small
            # deviation term instead of leaking the full mean into c.
            ssum = smallp.tile([8, 1], f32, tag="ssum", name=f"ssum_{bl}")
            nc.vector.reduce_sum(out=ssum, in_=ssum_parts, axis=AX.X)
            zs = smallp.tile([8, 1], f32, tag="zs", name=f"zs_{bl}")
            nc.vector.tensor_scalar_mul(zs, ssum, 1.0 / S)
            sinvS = smallp.tile([8, 1], f32, tag="sinvS", name=f"sinvS_{bl}")
            nc.vector.reciprocal(sinvS, zs)
            zT = smallp.tile([8, 1], f32, tag="zT", name=f"zT_{bl}")
            nc.vector.tensor_scalar_mul(zT, ssum, 1.0 / T)
            negb = smallp.tile([8, 1], f32, tag="negb", name=f"negb_{bl}")
            nc.vector.tensor_scalar(out=negb, in0=zT, scalar1=sinvS,
                                    scalar2=-1.0, op0=OP.mult, op1=OP.mult)
            wt = seqp.tile([8, T], bf16, tag="wt", name=f"wt_{bl}")
            # split across ACT and DVE: this sits on the tail critical path
            nc.scalar.activation(out=wt[:, :TH], in_=s_exp[:, :TH],
                                 func=AF.Identity, scale=sinvS, bias=negb)
            nc.vector.tensor_scalar(out=wt[:, TH:], in0=s_exp[:, TH:],
                                    scalar1=zT, scalar2=sinvS, op0=OP.subtract,
                                    op1=OP.mult)

            wt8Ts[bl] = wt

        def phase_b(bl):
            """wt transpose + deviation v-pass + output projection, batch bl"""
            hN = hNs[bl]
            wt = wt8Ts[bl]

            # ---- transpose w to [t, h] and cast fp8; by now wt is long
            # ---- ready, so the PE never stalls on the softmax chain ----
            psWT = psWTp.tile([128, T128, 8], bf16, tag="psWT", name=f"psWT_{bl}")
            for u in range(T128):
                nc.tensor.matmul(psWT[:, u, :],
                                 lhsT=wt[:, u * 128:(u + 1) * 128],
                                 rhs=ident[:8, :8], is_transpose=True,
                                 skip_group_check=True)
            wt8T = smallp.tile([128, T128, 8], f8, tag="wt8T", name=f"wt8T_{bl}")
            nc.vector.tensor_copy(out=wt8T, in_=psWT)

            # ---- deviation v^T[n, h] = sum_t hyp[t, n] w[t, h] ----
            # n outer: each psV accumulation group must fully complete before
            # the next group's start=True, which pending-zeroes the whole
            # 2KB PSUM region and would wipe other groups' partial sums.
            psV = psVp.tile([128, NCH, 8], f32, tag="psV", name=f"psV_{bl}")
            for n in range(NCH):
                nsl = slice(n * 128, (n + 1) * 128)
                for u in range(T128 // 2):
                    nc.tensor.matmul(psV[:, n, :],
                                     lhsT=hN[:, 2 * u:2 * u + 2, nsl],
                                     rhs=wt8T[:, 2 * u:2 * u + 2, :],
                                     start=(u == 0), stop=(u == T128 // 2 - 1),
                                     perf_mode=DR, skip_group_check=True)
            vT8 = smallp.tile([128, NCH, 8], f8, tag="vT8", name=f"vT8_{bl}")
            nc.scalar.activation(out=vT8[:, :NCH // 2, :],
                                 in_=psV[:, :NCH // 2, :], func=AF.Copy,
                                 scale=V_SCALE)
            nc.scalar.activation(out=vT8[:, NCH // 2:, :],
                                 in_=psV[:, NCH // 2:, :], func=AF.Copy,
                                 scale=V_SCALE)

            # ---- c_dev^T = Wmh v_dev ; c = c_dev/S + mbar ----
            # h outer for the same reason: complete each column's group first
            ps_c = psCp.tile([128, H], f32, tag="ps_c", name=f"ps_c_{bl}")
            for h in range(H):
                for n in range(NCH // 2):
                    nc.tensor.matmul(ps_c[:, h:h + 1],
                                     lhsT=wmhT[:, h, 2 * n:2 * n + 2, :],
                                     rhs=vT8[:, 2 * n:2 * n + 2, h:h + 1],
                                     start=(n == 0), stop=(n == NCH // 2 - 1),
                                     perf_mode=DR, skip_group_check=True)
            nc.vector.scalar_tensor_tensor(
                out=c2all[:, bl, :], in0=ps_c,
                scalar=1.0 / (S * WMH_SCALE * V_SCALE),
                in1=aux_sb[:, 2 * BL + bl * H:2 * BL + (bl + 1) * H],
                op0=OP.mult, op1=OP.add)

        # software pipeline: phase_b(bl) is emitted one batch behind so the
        # in-order PE queue never stalls on the transpose copies of batch bl
        for bl in range(BL):
            phase_a(bl)
            if bl >= 1:
                phase_b(bl - 1)
        phase_b(BL - 1)

        nc.sync.dma_start(out=out_d[:], in_=c2all)

    nc.compile()
    return nc


def _prep_inputs(hyp, Wmh, bmh, W, bW, Wm, bWm, Wh, bWh):
    """Host-side sharding + layout prep (numpy only)."""
    f8 = ml_dtypes.float8_e4m3
    bf = ml_dtypes.bfloat16
    hyp = np.asarray(hyp, np.float32)
    Wmh = np.asarray(Wmh, np.float32)
    bmh = np.asarray(bmh, np.float32)
    W = np.asarray(W, np.float32)
    bW = np.asarray(bW, np.float32)
    Wm = np.asarray(Wm, np.float32)
    bWm = np.asarray(bWm, np.float32)
    Wh = np.asarray(Wh, np.float32)

    hyp_b = np.ascontiguousarray(hyp.transpose(1, 0, 2))          # (B, T, N)
    hypN_all = hyp_b.astype(f8)
    hypT_all = np.ascontiguousarray(hyp_b.transpose(0, 2, 1)).astype(f8)

    # fused scoring weights: WS[h*16+q, n] = sum_k W[q,k] Wmh[h,k,n]
    WS = np.einsum('qk,hkn->hqn', W, Wmh).reshape(128, N)
    bSp = (np.einsum('qk,hk->hq', W, bmh).reshape(128)
           + np.tile(bW, H)).astype(np.float32)
    WSm = np.einsum('qk,hkn->hqn', Wm, Wmh).reshape(128, N)
    bSm = (np.einsum('qk,hk->hq', Wm, bmh).reshape(128)
           + np.tile(bWm, H)).astype(np.float32)

    # per-batch time-mean and everything that depends only on it (host f32)
    xbar = np.asarray(hyp_b.mean(axis=1, dtype=np.float64), np.float32)
    sbias_all = (xbar @ WS.T + bSp).astype(np.float32)            # (B, 128)
    mw_all = np.tanh(xbar @ WSm.T + bSm).astype(np.float32)       # (B, 128)
    mbar_all = (np.einsum('bn,hkn->bhk', xbar, Wmh)
                + bmh).astype(np.float32)                         # (B, H, K)

    # WST (128, NCH, 128): [p, c, q] = WS_SCALE * WS[q, c*128+p]
    WST = np.ascontiguousarray(
        (WS * WS_SCALE).T.reshape(NCH, 128, 128).transpose(1, 0, 2)).astype(f8)
    # Wmh (H, K, N) -> (128, H, NCH, K): [p, h, c, k] = WMH_SCALE*Wmh[h,k,c*128+p]
    wmhT = np.ascontiguousarray(
        (Wmh * WMH_SCALE).transpose(2, 0, 1).reshape(NCH, 128, H, K)
        .transpose(1, 2, 0, 3)).astype(f8)
    whD = np.zeros((K, H), dtype=np.float32)
    for h in range(H):
        whD[h * K2:(h + 1) * K2, h] = Wh
    whD = whD.astype(bf)

    in_maps = []
    for c in range(NCORES):
        sl = slice(c * BL, (c + 1) * BL)
        aux = np.empty((128, 2 * BL + BL * H), np.float32)
        aux[:, :BL] = sbias_all[sl].T
        aux[:, BL:2 * BL] = mw_all[sl].T
        # mbarT columns: [2BL + bl*H + h] = mbar[bl, h, :]
        aux[:, 2 * BL:] = mbar_all[sl].reshape(BL * H, K).T
        in_maps.append({
            "hypT": np.ascontiguousarray(hypT_all[sl]),
            "hypN": np.ascontiguousarray(hypN_all[sl]),
            "WST": WST, "whD": whD, "wmhT": wmhT,
            "aux": aux,
        })
    return in_maps


def kernel(hyp, Wmh, bmh, W, bW, Wm, bWm, Wh, bWh,
           dan_hidden_size=None, attention_hidden_size=None,
           multihead_size=None, **_):
    from concourse.bass_utils import run_bass_kernel_spmd

    in_maps = _prep_inputs(hyp, Wmh, bmh, W, bW, Wm, bWm, Wh, bWh)
    if "nc" not in _cache:
        _cache["nc"] = _build_nc()
    res = run_bass_kernel_spmd(_cache["nc"], in_maps, core_ids=list(range(NCORES)))
    # out is (K, BL, H) per core -> (BL, H, K) -> (BL, N)
    out = np.concatenate([r["out"].transpose(1, 2, 0).reshape(BL, N)
                          for r in res.results], axis=0)
    return out.astype(np.float32)
